# revision 13
# baseline (speedup 1.0000x reference)
"""GPT forward (L=6, B=2, T=1024, D=768, H=12, V=50257) on 8 TRN2 NeuronCores.

Sharding: tokens sharded 8-way (each core owns two causally-complementary
128-token blocks of one batch), weights replicated, per-layer K/V AllGather
within each 4-core batch group, classifier vocab-sharded 8-way after a final
hidden-state AllGather.  Activations are feature-major [D, t].

All matmul operands are bf16 (enables FWL fast weight load and full-rate
small matmuls); the residual stream and LN statistics stay fp32.  PSUM
evictions are spread across vector/gpsimd to keep the scalar engine free
for exp/gelu.  RoPE partition swaps run as SBUF->SBUF DMAs on four
different queues.  Collective payloads are bf16 (half the wire bytes).
The program is core-uniform: per-core differences (token positions, causal
masks, vocab slice) enter as input data.
"""
import os
import numpy as np
from contextlib import ExitStack

import concourse.bass as bass
import concourse.tile as tile
import concourse.mybir as mybir
from concourse import bacc, bass_utils
from concourse.masks import make_identity

F32 = mybir.dt.float32
F32R = mybir.dt.float32r
BF16 = mybir.dt.bfloat16
AF = mybir.ActivationFunctionType
OP = mybir.AluOpType

L, B, T, D, H, DK, V = 6, 2, 1024, 768, 12, 64, 50257
NB, TB, TPC = 8, 128, 256
NJ = D // 128                       # 6
NJ1 = 4 * D // 128                  # 24
VCHUNK = 512
NVC = 13
VCP = NVC * VCHUNK                  # 6656
VC = 6283                           # 8*6283 = 50264 >= V
EPS = 1e-5
NMT = 16
NLAYER = int(os.environ.get("KLAYERS", str(L)))

KB_RANK = [j if j < 4 else 7 - j for j in range(NB)]
KB_HALF = [0 if j < 4 else 1 for j in range(NB)]


def _build():
    nc = bacc.Bacc("TRN2", target_bir_lowering=False, debug=False)

    di = {}
    def din(name, shape, dt=F32R):
        di[name] = nc.dram_tensor(name, shape, dt, kind="ExternalInput")
        return di[name]

    din("x0T", [128, NJ * TPC])
    din("cosT", [128, NJ * TPC], BF16)
    din("sinS", [128, NJ * TPC], BF16)
    din("masks", [NB, 128, TPC], BF16)
    din("onecol", [128, 1])
    din("ones96", [128, NB * H, 2], BF16)
    din("embT", [D, VCP], BF16)
    for nm in ("Wq", "Wk", "Wv", "Wo"):
        din(nm, [L, D, D], BF16)
    din("W1", [L, D, 4 * D], BF16)
    din("W2", [L, 4 * D, D], BF16)
    for nm in ("bq_p", "bk_p", "bo_p", "b2_p", "g_p", "be_p", "l2w_p", "l2b_p"):
        din(nm, [L, 128, NJ], F32)
    din("b1_p", [L, 128, NJ1], F32)
    din("bv_bc", [L, 128, D], F32)
    din("lnw_p", [128, NJ], F32)
    din("lnb_p", [128, NJ], F32)

    out_logits = nc.dram_tensor("logits", [NMT * 128, VCP], BF16,
                                kind="ExternalOutput")

    with tile.TileContext(nc) as tc, ExitStack() as octx:
        const = octx.enter_context(tc.tile_pool(name="const", bufs=1))
        xpool = octx.enter_context(tc.tile_pool(name="x", bufs=1))
        small = octx.enter_context(tc.tile_pool(name="small", bufs=2))
        bias = octx.enter_context(tc.tile_pool(name="bias", bufs=2))
        pp = octx.enter_context(tc.tile_pool(name="pp", bufs=8, space="PSUM"))
        dram = octx.enter_context(tc.tile_pool(name="dram", bufs=2, space="DRAM"))

        t_ones = const.tile([128, 1], F32R, tag="ones")
        nc.sync.dma_start(t_ones[:], di["onecol"].ap())
        t_id = const.tile([128, 128], BF16, tag="ident")
        make_identity(nc, t_id[:])
        t_lnw = const.tile([128, NJ], F32, tag="lnw")
        nc.sync.dma_start(t_lnw[:], di["lnw_p"].ap())
        t_lnb = const.tile([128, NJ], F32, tag="lnb")
        nc.sync.dma_start(t_lnb[:], di["lnb_p"].ap())
        t_eps = const.tile([1, 1], F32, tag="eps")
        nc.gpsimd.memset(t_eps[:], EPS)

        t_x = xpool.tile([128, NJ * TPC], F32R, tag="x")
        nc.sync.dma_start(t_x[:], di["x0T"].ap())
        t_hT = xpool.tile([128, NJ * TPC], BF16, tag="hT")

        pcnt = [0]

        def psum(w=TPC):
            pcnt[0] += 1
            return pp.tile([128, w], F32, tag="pp", name=f"ps{pcnt[0]}")

        def psum1(w=TPC):
            pcnt[0] += 1
            return pp.tile([1, w], F32, tag="pp", name=f"ps{pcnt[0]}")

        def layernorm(wpool, src, dst, gt, bt, pre=None):
            """feature-major LN: dst(bf16) = (src - mean)/std * g + b.
            pre(j) runs before chunk j's stats (fused residual eviction)."""
            t_sq = wpool.tile([128, NJ * TPC], F32R, tag="scratch6")
            p_s = psum1()
            p_q = psum1()
            for j in range(NJ):
                if pre is not None:
                    pre(j)
                sl = slice(j * TPC, (j + 1) * TPC)
                eng = nc.vector if j % 2 == 0 else nc.gpsimd
                eng.tensor_tensor(t_sq[:, sl], src[:, sl], src[:, sl], OP.mult)
                nc.tensor.matmul(p_s[:], t_ones[:], src[:, sl],
                                 start=(j == 0), stop=(j == NJ - 1))
                nc.tensor.matmul(p_q[:], t_ones[:], t_sq[:, sl],
                                 start=(j == 0), stop=(j == NJ - 1))
            # ones vector holds 1/D, so p_s = mean and p_q = E[x^2] directly
            t_mean = small.tile([1, TPC], F32, tag="mean")
            nc.vector.tensor_copy(t_mean[:], p_s[:])
            t_msq = small.tile([1, TPC], F32, tag="msq")
            nc.vector.tensor_tensor(t_msq[:], t_mean[:], p_s[:], OP.mult)
            t_var = small.tile([1, TPC], F32, tag="var")
            nc.vector.tensor_tensor(t_var[:], p_q[:], t_msq[:], OP.subtract)
            t_std = small.tile([1, TPC], F32, tag="std")
            nc.scalar.activation(t_std[:], t_var[:], AF.Sqrt, bias=t_eps[:])
            t_rstd = small.tile([1, TPC], F32, tag="rstd")
            nc.vector.reciprocal(t_rstd[:], t_std[:])
            t_mb = small.tile([128, TPC], F32, tag="mb")
            nc.gpsimd.partition_broadcast(t_mb[:], t_mean[:])
            t_rb = small.tile([128, TPC], F32, tag="rb")
            nc.gpsimd.partition_broadcast(t_rb[:], t_rstd[:])
            t_c = wpool.tile([128, NJ * TPC], F32, tag="lnc")
            for j in range(NJ):
                sl = slice(j * TPC, (j + 1) * TPC)
                eng = nc.vector if j % 2 == 0 else nc.gpsimd
                eng.tensor_tensor(t_c[:, sl], src[:, sl], t_mb[:], OP.subtract)
                eng.tensor_tensor(dst[:, sl], t_c[:, sl], t_rb[:], OP.mult)
                eng.tensor_scalar(dst[:, sl], dst[:, sl], gt[:, j:j + 1],
                                  bt[:, j:j + 1], OP.mult, OP.add)

        def rope(wpool, t_q, t_cos, t_sin):
            """in-place RoPE on feature-major bf16 [128, NJ*TPC] tile.
            Partition swap via SBUF->SBUF DMAs on four queues."""
            t_sw = wpool.tile([128, NJ * TPC], BF16, tag="ropesw")
            W = NJ * TPC
            nc.sync.dma_start(t_sw[0:32, 0:W], t_q[32:64, 0:W])
            nc.scalar.dma_start(t_sw[32:64, 0:W], t_q[0:32, 0:W])
            nc.gpsimd.dma_start(t_sw[64:96, 0:W], t_q[96:128, 0:W])
            nc.sync.dma_start(t_sw[96:128, 0:W], t_q[64:96, 0:W])
            nc.gpsimd.tensor_tensor(t_sw[:], t_sw[:], t_sin[:], OP.mult)
            nc.vector.tensor_tensor(t_q[:], t_q[:], t_cos[:], OP.mult)
            nc.vector.tensor_tensor(t_q[:], t_q[:], t_sw[:], OP.add)

        def wpass(wsl_pool, wdram, l, nk, rhs, rhs_k_slice, out_fn):
            """out[n] = sum_k W[l,k].T @ rhs_k ; W streamed, psum-resident over n.
            out_fn(n, ps) evicts psum tile for output feature-tile n."""
            pss = [psum() for _ in range(NJ)]
            for k in range(nk):
                wk = wsl_pool.tile([128, NJ * 128], BF16, tag="wsl")
                nc.sync.dma_start(wk[:], wdram.ap()[l, k * 128:(k + 1) * 128, :])
                for n in range(NJ):
                    nc.tensor.matmul(pss[n][:], wk[:, n * 128:(n + 1) * 128],
                                     rhs[:, rhs_k_slice(k)],
                                     start=(k == 0), stop=(k == nk - 1))
            for n in range(NJ):
                out_fn(n, pss[n])

        def evict_bias(dst, dst_sl, bias_t, flip=0):
            """psum + bias -> bf16 sbuf, alternating vector / scalar."""
            def f(nn, p):
                if (nn + flip) % 2 == 0:
                    nc.vector.tensor_scalar(dst[:, dst_sl(nn)], p[:],
                                            bias_t[:, nn:nn + 1], None, OP.add)
                else:
                    nc.scalar.activation(dst[:, dst_sl(nn)], p[:], AF.Identity,
                                         bias=bias_t[:, nn:nn + 1])
            return f

        pend = [None]
        # ================= phase A: transformer layers =================
        with ExitStack() as actx:
            aconst = actx.enter_context(tc.tile_pool(name="aconst", bufs=1))
            kvp = actx.enter_context(tc.tile_pool(name="kvp", bufs=1))
            wk_ = actx.enter_context(tc.tile_pool(name="work", bufs=1))
            ap_ = actx.enter_context(tc.tile_pool(name="Ap", bufs=2))
            wsl = actx.enter_context(tc.tile_pool(name="wsl", bufs=6))
            h1p = actx.enter_context(tc.tile_pool(name="h1p", bufs=1))

            t_cos = aconst.tile([128, NJ * TPC], BF16, tag="cos")
            nc.sync.dma_start(t_cos[:], di["cosT"].ap())
            t_sin = aconst.tile([128, NJ * TPC], BF16, tag="sin")
            nc.sync.dma_start(t_sin[:], di["sinS"].ap())
            t_mask = aconst.tile([128, NB * TPC], BF16, tag="mask")
            for kb in range(NB):
                nc.sync.dma_start(t_mask[:, kb * TPC:(kb + 1) * TPC],
                                  di["masks"].ap()[kb])

            t_K = kvp.tile([128, NJ * NB * TB], BF16, tag="K")    # (j, kblk, t)
            t_V = kvp.tile([128, NB * H * 66], BF16, tag="V")     # (kblk, h, dk|one)
            nc.sync.dma_start(
                t_V[:].rearrange("p (b h e) -> p (b h) e", b=NB, h=H)[:, :, 64:66],
                di["ones96"].ap())

            for l in range(NLAYER):
                # --- per-layer bias/param tiles
                bt = {}
                for nm in ("bq_p", "bk_p", "bo_p", "b2_p", "g_p", "be_p",
                           "l2w_p", "l2b_p"):
                    bt[nm] = bias.tile([128, NJ], F32, tag=nm, name=f"bt_{nm}")
                    nc.sync.dma_start(bt[nm][:], di[nm].ap()[l])
                t_b1 = bias.tile([128, NJ1], F32, tag="b1")
                nc.sync.dma_start(t_b1[:], di["b1_p"].ap()[l])
                t_bvb = bias.tile([128, D], F32, tag="bvb")
                nc.sync.dma_start(t_bvb[:], di["bv_bc"].ap()[l])

                # --- LN1 (fused with the previous layer's W2 eviction+residual)
                t_xn = wk_.tile([128, NJ * TPC], BF16, tag="xn")
                layernorm(wk_, t_x, t_xn, bt["g_p"], bt["be_p"],
                          pre=pend[0])
                pend[0] = None

                # --- K projection (feature-major), RoPE, AllGather
                t_k = wk_.tile([128, NJ * TPC], BF16, tag="k")
                wpass(wsl, di["Wk"], l, NJ, t_xn,
                      lambda k: slice(k * TPC, (k + 1) * TPC),
                      evict_bias(t_k, lambda n: slice(n * TPC, (n + 1) * TPC),
                                 bt["bk_p"], 0))
                rope(wk_, t_k, t_cos, t_sin)
                # --- stage K into the merged K+V collective buffer
                kv_in = dram.tile([2 * D * TPC], BF16, tag="kv_in")
                nc.gpsimd.dma_start(
                    kv_in[0:D * TPC].rearrange("(j p t) -> p j t", j=NJ, p=128),
                    t_k[:].rearrange("p (j t) -> p j t", j=NJ))
                # --- V projection (token-major) + bias
                t_vc = wk_.tile([128, 2 * D], BF16, tag="vc")
                psv = [[psum(512), psum(256)] for _ in range(2)]
                for k in range(NJ):
                    wvk = wsl.tile([128, NJ * 128], BF16, tag="wsl")
                    nc.sync.dma_start(wvk[:], di["Wv"].ap()[l, k * 128:(k + 1) * 128, :])
                    for tt in range(2):
                        lhs = t_xn[:, k * TPC + tt * TB: k * TPC + (tt + 1) * TB]
                        nc.tensor.matmul(psv[tt][0][:], lhs, wvk[:, 0:512],
                                         start=(k == 0), stop=(k == NJ - 1))
                        nc.tensor.matmul(psv[tt][1][:], lhs, wvk[:, 512:768],
                                         start=(k == 0), stop=(k == NJ - 1))
                for tt in range(2):
                    nc.vector.tensor_tensor(
                        t_vc[:, tt * D: tt * D + 512], psv[tt][0][:],
                        t_bvb[:, 0:512], OP.add)
                    nc.vector.tensor_tensor(
                        t_vc[:, tt * D + 512: (tt + 1) * D], psv[tt][1][:],
                        t_bvb[:, 512:768], OP.add)

                nc.gpsimd.dma_start(
                    kv_in[D * TPC:].rearrange("(tt p e) -> p tt e", tt=2, p=128),
                    t_vc[:].rearrange("p (tt e) -> p tt e", tt=2))
                kv_out = dram.tile([4, 2 * D * TPC], BF16, tag="kv_out")
                nc.gpsimd.collective_compute(
                    "AllGather", OP.bypass,
                    replica_groups=[[0, 1, 2, 3], [4, 5, 6, 7]],
                    ins=[kv_in[:].opt()], outs=[kv_out[:].opt()])

                # --- Q projection + RoPE (overlaps the K/V collectives)
                t_q = wk_.tile([128, NJ * TPC], BF16, tag="q")
                wpass(wsl, di["Wq"], l, NJ, t_xn,
                      lambda k: slice(k * TPC, (k + 1) * TPC),
                      evict_bias(t_q, lambda n: slice(n * TPC, (n + 1) * TPC),
                                 bt["bq_p"], 1))
                rope(wk_, t_q, t_cos, t_sin)

                # --- load gathered K (feature-major) and V (token-major)
                # block slot order is (rank, half); masks are relabeled to match
                kk5 = t_K[:].rearrange("p (j r ft) -> p j r ft", j=NJ, r=4)
                vv5 = t_V[:].rearrange("p (r f h e) -> p r f h e", r=4, f=2, h=H)
                for r in range(4):
                    nc.scalar.dma_start(
                        kk5[:, :, r],
                        kv_out[r, 0:D * TPC]
                        .rearrange("(j p t) -> p j t", j=NJ, p=128))
                    for f in range(2):
                        nc.gpsimd.dma_start(
                            vv5[:, r, f, :, 0:64],
                            kv_out[r, D * TPC + f * (TB * D):
                                   D * TPC + (f + 1) * (TB * D)]
                            .rearrange("(p h e) -> p h e", p=128, h=H))

                # --- attention: head pairs share the PE via row groups
                t_att = wk_.tile([128, 2 * D], BF16, tag="att")   # (qi, h, dk)
                for hp in range(H // 2):
                    jq = hp                      # == h//2 for both heads
                    tA = [ap_.tile([128, NB * TPC], BF16, tag="A",
                                   name=f"A{l}_{hp}_{i}") for i in range(2)]
                    for kbp in range(4):
                        ps2 = [psum(512) for _ in range(2)]
                        for kk in range(2):
                            kb = 2 * kbp + kk
                            for hi in range(2):
                                po = 64 * hi
                                nc.tensor.matmul(
                                    ps2[hi][:, kk * TPC:(kk + 1) * TPC],
                                    t_K[po:po + 64,
                                        (jq * NB + kb) * TB:(jq * NB + kb + 1) * TB],
                                    t_q[po:po + 64, jq * TPC:(jq + 1) * TPC])
                        for hi in range(2):
                            asl = tA[hi][:, kbp * 512:(kbp + 1) * 512]
                            nc.scalar.activation(asl, ps2[hi][:], AF.Exp,
                                                 scale=0.125)
                            eng = nc.vector if (kbp + hi) % 2 == 0 else nc.gpsimd
                            eng.tensor_tensor(
                                asl, asl, t_mask[:, kbp * 512:(kbp + 1) * 512],
                                OP.mult)
                    for hi in range(2):
                        h = 2 * hp + hi
                        for qi in range(2):
                            pav = psum(66)
                            for kb in range(NB):
                                nc.tensor.matmul(
                                    pav[:],
                                    tA[hi][:, kb * TPC + qi * TB:
                                           kb * TPC + (qi + 1) * TB],
                                    t_V[:, (kb * H + h) * 66:(kb * H + h) * 66 + 66],
                                    start=(kb == 0), stop=(kb == NB - 1))
                            t_rl = small.tile([128, 1], F32, tag="rl")
                            nc.vector.reciprocal(t_rl[:], pav[:, 64:65])
                            osl = t_att[:, qi * D + h * 64: qi * D + (h + 1) * 64]
                            if (hi + qi) % 2 == 0:
                                nc.vector.tensor_scalar_mul(osl, pav[:, 0:64],
                                                            t_rl[:])
                            else:
                                nc.scalar.mul(osl, pav[:, 0:64], t_rl[:])

                # --- transpose att to feature-major
                t_attT = wk_.tile([128, NJ * TPC], BF16, tag="attT")
                for qi in range(2):
                    for j in range(NJ):
                        pcnt[0] += 1
                        ptr = pp.tile([128, 128], BF16, tag="pp",
                                      name=f"ps{pcnt[0]}")
                        nc.tensor.transpose(
                            ptr[:], t_att[:, qi * D + j * 128: qi * D + (j + 1) * 128],
                            t_id[:])
                        osl = t_attT[:, j * TPC + qi * TB: j * TPC + qi * TB + TB]
                        if (qi + j) % 2 == 0:
                            nc.vector.tensor_copy(osl, ptr[:])
                        else:
                            nc.scalar.copy(osl, ptr[:])

                # --- Wo + residual fused into LN2 chunk prologue
                t_mo = wk_.tile([128, NJ * TPC], F32, tag="mmout")
                po_ = [psum() for _ in range(NJ)]
                for k in range(NJ):
                    wok = wsl.tile([128, NJ * 128], BF16, tag="wsl")
                    nc.sync.dma_start(wok[:],
                                      di["Wo"].ap()[l, k * 128:(k + 1) * 128, :])
                    for n in range(NJ):
                        nc.tensor.matmul(po_[n][:], wok[:, n * 128:(n + 1) * 128],
                                         t_attT[:, k * TPC:(k + 1) * TPC],
                                         start=(k == 0), stop=(k == NJ - 1))
                eb_o = evict_bias(t_mo, lambda n: slice(n * TPC, (n + 1) * TPC),
                                  bt["bo_p"], 0)

                def pre_o(j):
                    sl = slice(j * TPC, (j + 1) * TPC)
                    eb_o(j, po_[j])
                    nc.gpsimd.tensor_tensor(t_x[:, sl], t_x[:, sl], t_mo[:, sl],
                                            OP.add)

                # --- LN2 + MLP
                t_xn2 = wk_.tile([128, NJ * TPC], BF16, tag="xn")
                layernorm(wk_, t_x, t_xn2, bt["l2w_p"], bt["l2b_p"], pre=pre_o)

                t_h1 = h1p.tile([128, NJ1 * TPC], BF16, tag="h1")
                for g in range(4):
                    psg = [psum() for _ in range(NJ)]
                    for k in range(NJ):
                        w1k = wsl.tile([128, NJ * 128], BF16, tag="wsl")
                        nc.sync.dma_start(
                            w1k[:], di["W1"].ap()[l, k * 128:(k + 1) * 128,
                                                  g * D:(g + 1) * D])
                        for n in range(NJ):
                            nc.tensor.matmul(
                                psg[n][:], w1k[:, n * 128:(n + 1) * 128],
                                t_xn2[:, k * TPC:(k + 1) * TPC],
                                start=(k == 0), stop=(k == NJ - 1))
                    for n in range(NJ):
                        gn = g * NJ + n
                        nc.scalar.activation(
                            t_h1[:, gn * TPC:(gn + 1) * TPC], psg[n][:], AF.Gelu,
                            bias=t_b1[:, gn:gn + 1])

                p2_ = [psum() for _ in range(NJ)]
                for k in range(NJ1):
                    w2k = wsl.tile([128, NJ * 128], BF16, tag="wsl")
                    nc.sync.dma_start(w2k[:],
                                      di["W2"].ap()[l, k * 128:(k + 1) * 128, :])
                    for n in range(NJ):
                        nc.tensor.matmul(p2_[n][:], w2k[:, n * 128:(n + 1) * 128],
                                         t_h1[:, k * TPC:(k + 1) * TPC],
                                         start=(k == 0), stop=(k == NJ1 - 1))
                eb_2 = evict_bias(t_mo, lambda n: slice(n * TPC, (n + 1) * TPC),
                                  bt["b2_p"], 1)

                def mk_pre2(psums, eb, mo):
                    def pre2(j):
                        sl = slice(j * TPC, (j + 1) * TPC)
                        eb(j, psums[j])
                        nc.gpsimd.tensor_tensor(t_x[:, sl], t_x[:, sl],
                                                mo[:, sl], OP.add)
                    return pre2
                pend[0] = mk_pre2(p2_, eb_2, t_mo)

        # ================= phase B: final LN + classifier =================
        with ExitStack() as bctx:
            bw = bctx.enter_context(tc.tile_pool(name="bw", bufs=1))
            hallp = bctx.enter_context(tc.tile_pool(name="hall", bufs=1))
            embp = bctx.enter_context(tc.tile_pool(name="embp", bufs=14))

            layernorm(bw, t_x, t_hT, t_lnw, t_lnb, pre=pend[0])
            pend[0] = None
            hag_in = dram.tile([D, TPC], BF16, tag="hag_in")
            nc.gpsimd.dma_start(
                hag_in[:].rearrange("(j p) t -> p j t", p=128),
                t_hT[:].rearrange("p (j t) -> p j t", j=NJ))
            hag_out = dram.tile([8 * D, TPC], BF16, tag="hag_out",
                                addr_space="Shared")
            nc.gpsimd.collective_compute(
                "AllGather", OP.bypass,
                replica_groups=[[0, 1, 2, 3, 4, 5, 6, 7]],
                ins=[hag_in[:].opt()], outs=[hag_out[:].opt()])

            t_hall = hallp.tile([128, 8 * NJ * TPC], BF16, tag="hall")
            hall4 = t_hall[:].rearrange("p (r j t) -> p r j t", r=8, j=NJ)
            nc.scalar.dma_start(
                hall4[:, 0:4], hag_out[0:4 * D, :]
                .rearrange("(r j p) t -> p r j t", r=4, p=128))
            nc.gpsimd.dma_start(
                hall4[:, 4:8], hag_out[4 * D:8 * D, :]
                .rearrange("(r j p) t -> p r j t", r=4, p=128))

            for vc in range(NVC):
                ets = []
                for k in range(NJ):
                    et = embp.tile([128, VCHUNK], BF16, tag="emb", name=f"emb{vc}_{k}")
                    nc.sync.dma_start(
                        et[:], di["embT"].ap()[k * 128:(k + 1) * 128,
                                               vc * VCHUNK:(vc + 1) * VCHUNK])
                    ets.append(et)
                for mt in range(NMT):
                    beta, j = divmod(mt, NB)
                    r, hf = beta * 4 + KB_RANK[j], KB_HALF[j]
                    pc = psum(VCHUNK)
                    for k in range(NJ):
                        nc.tensor.matmul(
                            pc[:],
                            t_hall[:, (r * NJ + k) * TPC + hf * TB:
                                   (r * NJ + k) * TPC + (hf + 1) * TB],
                            ets[k][:], start=(k == 0), stop=(k == NJ - 1))
                    so = embp.tile([128, VCHUNK], BF16, tag="clso",
                                   name=f"clso{vc}_{mt}")
                    if mt % 2 == 0:
                        nc.scalar.copy(so[:], pc[:])
                    else:
                        nc.vector.tensor_copy(so[:], pc[:])
                    nc.gpsimd.dma_start(
                        out_logits.ap()[mt * 128:(mt + 1) * 128,
                                        vc * VCHUNK:(vc + 1) * VCHUNK], so[:])

    nc.compile()
    return nc


_NC = None


def _get_nc():
    global _NC
    if _NC is None:
        _NC = _build()
    return _NC


def _pack_fm(M):
    """[768, t] feature-major -> [128, 6*t] tile layout (row d=128*j+p)."""
    t = M.shape[1]
    return np.ascontiguousarray(
        M.reshape(NJ, 128, t).transpose(1, 0, 2).reshape(128, NJ * t),
        dtype=np.float32)


def _pack_pp(v):
    """per-feature vector [D'] -> per-partition [128, D'/128]."""
    return np.ascontiguousarray(v.reshape(-1, 128).T, dtype=np.float32)


def _prep_in_maps(inputs):
    import ml_dtypes
    bf = ml_dtypes.bfloat16
    f32 = lambda a: np.ascontiguousarray(a, dtype=np.float32)
    f16 = lambda a: np.ascontiguousarray(a, dtype=bf)
    emb = f32(inputs["emb"])
    tok = np.asarray(inputs["input_token"]).astype(np.int64)
    x0 = emb[tok]                                    # [B, T, D]

    shared = {
        "Wq": f16(inputs["Wq"]), "Wk": f16(inputs["Wk"]),
        "Wv": f16(inputs["Wv"]), "Wo": f16(inputs["Wo"]),
        "W1": f16(inputs["W1"]), "W2": f16(inputs["W2"]),
        "onecol": np.full((128, 1), 1.0 / D, np.float32),
        "ones96": np.ones((128, NB * H, 2), bf),
        "lnw_p": _pack_pp(f32(inputs["ln_w"])),
        "lnb_p": _pack_pp(f32(inputs["ln_b"])),
    }
    for nm, src in (("bq_p", "bq"), ("bk_p", "bk"), ("bo_p", "bo"),
                    ("b2_p", "b2"), ("g_p", "gamma"), ("be_p", "beta"),
                    ("l2w_p", "ln2_w"), ("l2b_p", "ln2_b")):
        shared[nm] = np.stack([_pack_pp(f32(inputs[src][l])) for l in range(L)])
    shared["b1_p"] = np.stack([_pack_pp(f32(inputs["b1"][l])) for l in range(L)])
    shared["bv_bc"] = np.stack(
        [np.tile(f32(inputs["bv"][l])[None, :], (128, 1)) for l in range(L)])

    # rope tables for one block-pair are built per core below
    inv = 1.0 / (10000.0 ** (np.arange(0, DK, 2, dtype=np.float32) / DK))
    embT_full = emb.T                                # [D, V]
    vpad = np.zeros((D, 8 * VC), np.float32)
    vpad[:, :V] = embT_full

    # diag causal mask (key-major): M[kt, qt] = 1 if kt <= qt
    diag = np.tril(np.ones((TB, TB), np.float32)).T

    in_maps = []
    for c in range(8):
        beta, i = divmod(c, 4)
        qb = (i, 7 - i)
        pos = np.concatenate([np.arange(qb[0] * TB, (qb[0] + 1) * TB),
                              np.arange(qb[1] * TB, (qb[1] + 1) * TB)])
        xc = x0[beta, pos]                           # [256, D]
        m = dict(shared)
        m["x0T"] = _pack_fm(xc.T)

        fr = pos[:, None].astype(np.float32) * inv[None, :]      # [256, 32]
        ang = np.concatenate([fr, fr], 1)                        # [256, 64]
        cosT = np.cos(ang).T                                     # [64, 256]
        sinT = np.sin(ang).T
        sinSg = sinT.copy()
        sinSg[:32] = -sinT[:32]
        m["cosT"] = np.ascontiguousarray(np.tile(cosT, (2, NJ))).astype(bf)
        m["sinS"] = np.ascontiguousarray(np.tile(sinSg, (2, NJ))).astype(bf)

        # block slots ordered (rank, half): slot 2r+f covers global block
        # r (f=0) or 7-r (f=1)
        masks = np.zeros((NB, 128, TPC), np.float32)
        for r4 in range(4):
            for f in range(2):
                kbg = r4 if f == 0 else 7 - r4
                slot = 2 * r4 + f
                for qi in range(2):
                    blk = qb[qi]
                    if kbg < blk:
                        masks[slot, :, qi * TB:(qi + 1) * TB] = 1.0
                    elif kbg == blk:
                        masks[slot, :, qi * TB:(qi + 1) * TB] = diag
        m["masks"] = masks.astype(bf)

        esl = np.zeros((D, VCP), np.float32)
        esl[:, :VC] = vpad[:, c * VC:(c + 1) * VC]
        m["embT"] = esl.astype(bf)
        in_maps.append(m)

    return in_maps


def _assemble(res):
    out = np.empty((B, T, 8 * VC), np.float32)
    for c in range(8):
        lr = np.asarray(res.results[c]["logits"]).astype(np.float32)
        lr = lr.reshape(B, T, VCP)
        out[:, :, c * VC:(c + 1) * VC] = lr[:, :, :VC]
    return np.ascontiguousarray(out[:, :, :V])


def kernel(**inputs):
    nc = _get_nc()
    in_maps = _prep_in_maps(inputs)
    res = bass_utils.run_bass_kernel_spmd(nc, in_maps, core_ids=list(range(8)))
    return _assemble(res)


def run_traced(inputs, tmpdir):
    nc = _get_nc()
    in_maps = _prep_in_maps(inputs)
    return bass_utils.run_bass_kernel_spmd(
        nc, in_maps, core_ids=list(range(8)), trace=True, tmpdir=tmpdir)


# revision 16
# speedup vs baseline: 1.1778x; 1.1778x over previous
"""GPT forward (L=6, B=2, T=1024, D=768, H=12, V=50257) on 8 TRN2 NeuronCores.

Sharding: tokens sharded 8-way (each core owns two causally-complementary
128-token blocks of one batch), weights replicated, per-layer K/V AllGather
within each 4-core batch group, classifier vocab-sharded 8-way after a final
hidden-state AllGather.  Activations are feature-major [D, t].

All matmul operands are bf16 (enables FWL fast weight load and full-rate
small matmuls); the residual stream and LN statistics stay fp32.  PSUM
evictions are spread across vector/gpsimd to keep the scalar engine free
for exp/gelu.  RoPE partition swaps run as SBUF->SBUF DMAs on four
different queues.  Collective payloads are bf16 (half the wire bytes).
The program is core-uniform: per-core differences (token positions, causal
masks, vocab slice) enter as input data.
"""
import os
import numpy as np
from contextlib import ExitStack

import concourse.bass as bass
import concourse.tile as tile
import concourse.mybir as mybir
from concourse import bacc, bass_utils
from concourse.masks import make_identity

F32 = mybir.dt.float32
F32R = mybir.dt.float32r
BF16 = mybir.dt.bfloat16
AF = mybir.ActivationFunctionType
OP = mybir.AluOpType

L, B, T, D, H, DK, V = 6, 2, 1024, 768, 12, 64, 50257
NB, TB, TPC = 8, 128, 256
NJ = D // 128                       # 6
NJ1 = 4 * D // 128                  # 24
VCHUNK = 512
NVC = 13
VCP = NVC * VCHUNK                  # 6656
VC = 6283                           # 8*6283 = 50264 >= V
EPS = 1e-5
NMT = 16
NLAYER = int(os.environ.get("KLAYERS", str(L)))

KB_RANK = [j if j < 4 else 7 - j for j in range(NB)]
KB_HALF = [0 if j < 4 else 1 for j in range(NB)]


def _build():
    nc = bacc.Bacc("TRN2", target_bir_lowering=False, debug=False)

    di = {}
    def din(name, shape, dt=F32R):
        di[name] = nc.dram_tensor(name, shape, dt, kind="ExternalInput")
        return di[name]

    din("x0T", [128, NJ * TPC])
    din("cosT", [128, NJ * TPC], BF16)
    din("sinS", [128, NJ * TPC], BF16)
    din("masks", [NB, 128, TPC], BF16)
    din("onecol", [128, 1])
    din("onesrow", [1, 128])
    din("ones96", [128, NB * H, 2], BF16)
    din("embT", [D, VCP], BF16)
    for nm in ("Wq", "Wk", "Wv", "Wo"):
        din(nm, [L, D, D], BF16)
    din("W1", [L, D, 4 * D], BF16)
    din("W2", [L, 4 * D, D], BF16)
    for nm in ("bq_p", "bk_p", "bo_p", "b2_p", "g_p", "be_p", "l2w_p", "l2b_p"):
        din(nm, [L, 128, NJ], F32)
    din("b1_p", [L, 128, NJ1], F32)
    din("bv_bc", [L, 128, D], F32)
    din("lnw_p", [128, NJ], F32)
    din("lnb_p", [128, NJ], F32)

    out_logits = nc.dram_tensor("logits", [NMT * 128, VCP], BF16,
                                kind="ExternalOutput")

    with tile.TileContext(nc) as tc, ExitStack() as octx:
        const = octx.enter_context(tc.tile_pool(name="const", bufs=1))
        xpool = octx.enter_context(tc.tile_pool(name="x", bufs=1))
        small = octx.enter_context(tc.tile_pool(name="small", bufs=2))
        bias = octx.enter_context(tc.tile_pool(name="bias", bufs=2))
        pp = octx.enter_context(tc.tile_pool(name="pp", bufs=8, space="PSUM"))
        dram = octx.enter_context(tc.tile_pool(name="dram", bufs=2, space="DRAM"))

        t_ones = const.tile([128, 1], F32R, tag="ones")
        nc.sync.dma_start(t_ones[:], di["onecol"].ap())
        t_onesr = const.tile([1, 128], F32R, tag="onesr")
        nc.sync.dma_start(t_onesr[:], di["onesrow"].ap())
        t_id = const.tile([128, 128], BF16, tag="ident")
        make_identity(nc, t_id[:])
        t_lnw = const.tile([128, NJ], F32, tag="lnw")
        nc.sync.dma_start(t_lnw[:], di["lnw_p"].ap())
        t_lnb = const.tile([128, NJ], F32, tag="lnb")
        nc.sync.dma_start(t_lnb[:], di["lnb_p"].ap())
        t_eps = const.tile([1, 1], F32, tag="eps")
        nc.gpsimd.memset(t_eps[:], EPS)

        t_x = xpool.tile([128, NJ * TPC], F32R, tag="x")
        nc.sync.dma_start(t_x[:], di["x0T"].ap())
        t_hT = xpool.tile([128, NJ * TPC], BF16, tag="hT")

        pcnt = [0]

        def psum(w=TPC):
            pcnt[0] += 1
            return pp.tile([128, w], F32, tag="pp", name=f"ps{pcnt[0]}")

        def psum1(w=TPC):
            pcnt[0] += 1
            return pp.tile([1, w], F32, tag="pp", name=f"ps{pcnt[0]}")

        def layernorm(wpool, src, dst, gt, bt, pre=None):
            """feature-major LN: dst(bf16) = (src - mean)/std * g + b.
            pre(j) runs before chunk j's stats (fused residual eviction)."""
            t_sq = wpool.tile([128, NJ * TPC], F32R, tag="scratch6")
            p_s = psum1()
            p_q = psum1()
            for j in range(NJ):
                if pre is not None:
                    pre(j)
                sl = slice(j * TPC, (j + 1) * TPC)
                nc.gpsimd.tensor_tensor(t_sq[:, sl], src[:, sl], src[:, sl],
                                        OP.mult)
                nc.tensor.matmul(p_s[:], t_ones[:], src[:, sl],
                                 start=(j == 0), stop=(j == NJ - 1))
                nc.tensor.matmul(p_q[:], t_ones[:], t_sq[:, sl],
                                 start=(j == 0), stop=(j == NJ - 1))
            # ones vector holds 1/D, so p_s = mean and p_q = E[x^2] directly
            t_mean = small.tile([1, TPC], F32R, tag="mean")
            nc.vector.tensor_copy(t_mean[:], p_s[:])
            t_msq = small.tile([1, TPC], F32, tag="msq")
            nc.vector.tensor_tensor(t_msq[:], t_mean[:], p_s[:], OP.mult)
            t_var = small.tile([1, TPC], F32, tag="var")
            nc.vector.tensor_tensor(t_var[:], p_q[:], t_msq[:], OP.subtract)
            t_std = small.tile([1, TPC], F32, tag="std")
            nc.scalar.activation(t_std[:], t_var[:], AF.Sqrt, bias=t_eps[:])
            t_rstd = small.tile([1, TPC], F32R, tag="rstd")
            with nc.allow_low_precision(reason="f32r view of f32 for PE bcast"):
                nc.vector.reciprocal(t_rstd[:], t_std[:])
            t_mb = small.tile([128, TPC], F32, tag="mb")
            p_mb = psum()
            nc.tensor.matmul(p_mb[:], t_onesr[:], t_mean[:])
            nc.vector.tensor_copy(t_mb[:], p_mb[:])
            t_rb = small.tile([128, TPC], F32, tag="rb")
            p_rb = psum()
            nc.tensor.matmul(p_rb[:], t_onesr[:], t_rstd[:])
            nc.vector.tensor_copy(t_rb[:], p_rb[:])
            t_c = wpool.tile([128, NJ * TPC], F32, tag="lnc")
            for j in range(NJ):
                sl = slice(j * TPC, (j + 1) * TPC)
                eng = nc.vector if j % 2 == 0 else nc.gpsimd
                eng.tensor_tensor(t_c[:, sl], src[:, sl], t_mb[:], OP.subtract)
                eng.tensor_tensor(dst[:, sl], t_c[:, sl], t_rb[:], OP.mult)
                eng.tensor_scalar(dst[:, sl], dst[:, sl], gt[:, j:j + 1],
                                  bt[:, j:j + 1], OP.mult, OP.add)

        def rope(wpool, t_q, t_cos, t_sin):
            """in-place RoPE on feature-major bf16 [128, NJ*TPC] tile.
            Partition swap via SBUF->SBUF DMAs on four queues."""
            t_sw = wpool.tile([128, NJ * TPC], BF16, tag="ropesw")
            W = NJ * TPC
            nc.sync.dma_start(t_sw[0:32, 0:W], t_q[32:64, 0:W])
            nc.scalar.dma_start(t_sw[32:64, 0:W], t_q[0:32, 0:W])
            nc.gpsimd.dma_start(t_sw[64:96, 0:W], t_q[96:128, 0:W])
            nc.sync.dma_start(t_sw[96:128, 0:W], t_q[64:96, 0:W])
            nc.gpsimd.tensor_tensor(t_sw[:], t_sw[:], t_sin[:], OP.mult)
            nc.vector.tensor_tensor(t_q[:], t_q[:], t_cos[:], OP.mult)
            nc.vector.tensor_tensor(t_q[:], t_q[:], t_sw[:], OP.add)

        def wpass(wsl_pool, wdram, l, nk, rhs, rhs_k_slice, out_fn):
            """out[n] = sum_k W[l,k].T @ rhs_k ; W streamed, psum-resident over n.
            out_fn(n, ps) evicts psum tile for output feature-tile n."""
            pss = [psum() for _ in range(NJ)]
            for k in range(nk):
                wk = wsl_pool.tile([128, NJ * 128], BF16, tag="wsl")
                nc.sync.dma_start(wk[:], wdram.ap()[l, k * 128:(k + 1) * 128, :])
                for n in range(NJ):
                    nc.tensor.matmul(pss[n][:], wk[:, n * 128:(n + 1) * 128],
                                     rhs[:, rhs_k_slice(k)],
                                     start=(k == 0), stop=(k == nk - 1))
            for n in range(NJ):
                out_fn(n, pss[n])

        def evict_bias(dst, dst_sl, bias_t, flip=0):
            """psum + bias -> bf16 sbuf, alternating vector / scalar."""
            def f(nn, p):
                if (nn + flip) % 2 == 0:
                    nc.vector.tensor_scalar(dst[:, dst_sl(nn)], p[:],
                                            bias_t[:, nn:nn + 1], None, OP.add)
                else:
                    nc.scalar.activation(dst[:, dst_sl(nn)], p[:], AF.Identity,
                                         bias=bias_t[:, nn:nn + 1])
            return f

        pend = [None]
        # ================= phase A: transformer layers =================
        with ExitStack() as actx:
            aconst = actx.enter_context(tc.tile_pool(name="aconst", bufs=1))
            kvp = actx.enter_context(tc.tile_pool(name="kvp", bufs=1))
            wk_ = actx.enter_context(tc.tile_pool(name="work", bufs=1))
            ap_ = actx.enter_context(tc.tile_pool(name="Ap", bufs=2))
            wsl = actx.enter_context(tc.tile_pool(name="wsl", bufs=6))
            h1p = actx.enter_context(tc.tile_pool(name="h1p", bufs=1))

            t_cos = aconst.tile([128, NJ * TPC], BF16, tag="cos")
            nc.sync.dma_start(t_cos[:], di["cosT"].ap())
            t_sin = aconst.tile([128, NJ * TPC], BF16, tag="sin")
            nc.sync.dma_start(t_sin[:], di["sinS"].ap())
            t_mask = aconst.tile([128, NB * TPC], BF16, tag="mask")
            for kb in range(NB):
                nc.sync.dma_start(t_mask[:, kb * TPC:(kb + 1) * TPC],
                                  di["masks"].ap()[kb])

            t_K = kvp.tile([128, NJ * NB * TB], BF16, tag="K")    # (j, kblk, t)
            t_V = kvp.tile([128, NB * H * 66], BF16, tag="V")     # (kblk, h, dk|one)
            nc.sync.dma_start(
                t_V[:].rearrange("p (b h e) -> p (b h) e", b=NB, h=H)[:, :, 64:66],
                di["ones96"].ap())

            for l in range(NLAYER):
                # --- per-layer bias/param tiles
                bt = {}
                for nm in ("bq_p", "bk_p", "bo_p", "b2_p", "g_p", "be_p",
                           "l2w_p", "l2b_p"):
                    bt[nm] = bias.tile([128, NJ], F32, tag=nm, name=f"bt_{nm}")
                    nc.sync.dma_start(bt[nm][:], di[nm].ap()[l])
                t_b1 = bias.tile([128, NJ1], F32, tag="b1")
                nc.sync.dma_start(t_b1[:], di["b1_p"].ap()[l])
                t_bvb = bias.tile([128, D], F32, tag="bvb")
                nc.sync.dma_start(t_bvb[:], di["bv_bc"].ap()[l])

                # --- LN1 (fused with the previous layer's W2 eviction+residual)
                t_xn = wk_.tile([128, NJ * TPC], BF16, tag="xn")
                layernorm(wk_, t_x, t_xn, bt["g_p"], bt["be_p"],
                          pre=pend[0])
                pend[0] = None

                # --- K projection (feature-major), RoPE, AllGather
                t_k = wk_.tile([128, NJ * TPC], BF16, tag="k")
                wpass(wsl, di["Wk"], l, NJ, t_xn,
                      lambda k: slice(k * TPC, (k + 1) * TPC),
                      evict_bias(t_k, lambda n: slice(n * TPC, (n + 1) * TPC),
                                 bt["bk_p"], 0))
                rope(wk_, t_k, t_cos, t_sin)
                # --- stage K into the merged K+V collective buffer
                kv_in = dram.tile([2 * D * TPC], BF16, tag="kv_in")
                nc.scalar.dma_start(
                    kv_in[0:D * TPC].rearrange("(j p t) -> p j t", j=NJ, p=128),
                    t_k[:].rearrange("p (j t) -> p j t", j=NJ))
                # --- V projection (token-major) + bias
                t_vc = wk_.tile([128, 2 * D], BF16, tag="vc")
                psv = [[psum(512), psum(256)] for _ in range(2)]
                for k in range(NJ):
                    wvk = wsl.tile([128, NJ * 128], BF16, tag="wsl")
                    nc.sync.dma_start(wvk[:], di["Wv"].ap()[l, k * 128:(k + 1) * 128, :])
                    for tt in range(2):
                        lhs = t_xn[:, k * TPC + tt * TB: k * TPC + (tt + 1) * TB]
                        nc.tensor.matmul(psv[tt][0][:], lhs, wvk[:, 0:512],
                                         start=(k == 0), stop=(k == NJ - 1))
                        nc.tensor.matmul(psv[tt][1][:], lhs, wvk[:, 512:768],
                                         start=(k == 0), stop=(k == NJ - 1))
                for tt in range(2):
                    nc.vector.tensor_tensor(
                        t_vc[:, tt * D: tt * D + 512], psv[tt][0][:],
                        t_bvb[:, 0:512], OP.add)
                    nc.vector.tensor_tensor(
                        t_vc[:, tt * D + 512: (tt + 1) * D], psv[tt][1][:],
                        t_bvb[:, 512:768], OP.add)

                nc.scalar.dma_start(
                    kv_in[D * TPC:].rearrange("(tt p e) -> p tt e", tt=2, p=128),
                    t_vc[:].rearrange("p (tt e) -> p tt e", tt=2))
                kv_out = dram.tile([4, 2 * D * TPC], BF16, tag="kv_out")
                nc.gpsimd.collective_compute(
                    "AllGather", OP.bypass,
                    replica_groups=[[0, 1, 2, 3], [4, 5, 6, 7]],
                    ins=[kv_in[:].opt()], outs=[kv_out[:].opt()])

                # --- Q projection + RoPE (overlaps the K/V collectives)
                t_q = wk_.tile([128, NJ * TPC], BF16, tag="q")
                wpass(wsl, di["Wq"], l, NJ, t_xn,
                      lambda k: slice(k * TPC, (k + 1) * TPC),
                      evict_bias(t_q, lambda n: slice(n * TPC, (n + 1) * TPC),
                                 bt["bq_p"], 1))
                rope(wk_, t_q, t_cos, t_sin)

                # --- load gathered K (feature-major) and V (token-major)
                # block slot order is (rank, half); masks are relabeled to match
                kk5 = t_K[:].rearrange("p (j r ft) -> p j r ft", j=NJ, r=4)
                vv5 = t_V[:].rearrange("p (r f h e) -> p r f h e", r=4, f=2, h=H)
                for r in range(4):
                    nc.scalar.dma_start(
                        kk5[:, :, r],
                        kv_out[r, 0:D * TPC]
                        .rearrange("(j p t) -> p j t", j=NJ, p=128))
                    for f in range(2):
                        nc.gpsimd.dma_start(
                            vv5[:, r, f, :, 0:64],
                            kv_out[r, D * TPC + f * (TB * D):
                                   D * TPC + (f + 1) * (TB * D)]
                            .rearrange("(p h e) -> p h e", p=128, h=H))

                # --- attention: head pairs share the PE via row groups
                t_att = wk_.tile([128, 2 * D], BF16, tag="att")   # (qi, h, dk)
                for hp in range(H // 2):
                    jq = hp                      # == h//2 for both heads
                    tA = [ap_.tile([128, NB * TPC], BF16, tag="A",
                                   name=f"A{l}_{hp}_{i}") for i in range(2)]
                    for kbp in range(4):
                        ps2 = [psum(512) for _ in range(2)]
                        for kk in range(2):
                            kb = 2 * kbp + kk
                            for hi in range(2):
                                po = 64 * hi
                                nc.tensor.matmul(
                                    ps2[hi][:, kk * TPC:(kk + 1) * TPC],
                                    t_K[po:po + 64,
                                        (jq * NB + kb) * TB:(jq * NB + kb + 1) * TB],
                                    t_q[po:po + 64, jq * TPC:(jq + 1) * TPC])
                        for hi in range(2):
                            asl = tA[hi][:, kbp * 512:(kbp + 1) * 512]
                            nc.scalar.activation(asl, ps2[hi][:], AF.Exp,
                                                 scale=0.125)
                            eng = nc.vector if (kbp + hi) % 2 == 0 else nc.gpsimd
                            eng.tensor_tensor(
                                asl, asl, t_mask[:, kbp * 512:(kbp + 1) * 512],
                                OP.mult)
                    for hi in range(2):
                        h = 2 * hp + hi
                        for qi in range(2):
                            pav = psum(66)
                            for kb in range(NB):
                                nc.tensor.matmul(
                                    pav[:],
                                    tA[hi][:, kb * TPC + qi * TB:
                                           kb * TPC + (qi + 1) * TB],
                                    t_V[:, (kb * H + h) * 66:(kb * H + h) * 66 + 66],
                                    start=(kb == 0), stop=(kb == NB - 1))
                            t_rl = small.tile([128, 1], F32, tag="rl")
                            nc.vector.reciprocal(t_rl[:], pav[:, 64:65])
                            osl = t_att[:, qi * D + h * 64: qi * D + (h + 1) * 64]
                            if (hi + qi) % 2 == 0:
                                nc.vector.tensor_scalar_mul(osl, pav[:, 0:64],
                                                            t_rl[:])
                            else:
                                nc.scalar.mul(osl, pav[:, 0:64], t_rl[:])

                # --- transpose att to feature-major
                t_attT = wk_.tile([128, NJ * TPC], BF16, tag="attT")
                for qi in range(2):
                    for j in range(NJ):
                        pcnt[0] += 1
                        ptr = pp.tile([128, 128], BF16, tag="pp",
                                      name=f"ps{pcnt[0]}")
                        nc.tensor.transpose(
                            ptr[:], t_att[:, qi * D + j * 128: qi * D + (j + 1) * 128],
                            t_id[:])
                        osl = t_attT[:, j * TPC + qi * TB: j * TPC + qi * TB + TB]
                        if (qi + j) % 2 == 0:
                            nc.vector.tensor_copy(osl, ptr[:])
                        else:
                            nc.scalar.copy(osl, ptr[:])

                # --- Wo + residual fused into LN2 chunk prologue
                t_mo = wk_.tile([128, NJ * TPC], F32, tag="mmout")
                po_ = [psum() for _ in range(NJ)]
                for k in range(NJ):
                    wok = wsl.tile([128, NJ * 128], BF16, tag="wsl")
                    nc.sync.dma_start(wok[:],
                                      di["Wo"].ap()[l, k * 128:(k + 1) * 128, :])
                    for n in range(NJ):
                        nc.tensor.matmul(po_[n][:], wok[:, n * 128:(n + 1) * 128],
                                         t_attT[:, k * TPC:(k + 1) * TPC],
                                         start=(k == 0), stop=(k == NJ - 1))
                eb_o = evict_bias(t_mo, lambda n: slice(n * TPC, (n + 1) * TPC),
                                  bt["bo_p"], 0)

                def pre_o(j):
                    sl = slice(j * TPC, (j + 1) * TPC)
                    eb_o(j, po_[j])
                    nc.vector.tensor_tensor(t_x[:, sl], t_x[:, sl], t_mo[:, sl],
                                            OP.add)

                # --- LN2 + MLP
                t_xn2 = wk_.tile([128, NJ * TPC], BF16, tag="xn")
                layernorm(wk_, t_x, t_xn2, bt["l2w_p"], bt["l2b_p"], pre=pre_o)

                t_h1 = h1p.tile([128, NJ1 * TPC], BF16, tag="h1")
                for g in range(4):
                    psg = [psum() for _ in range(NJ)]
                    for k in range(NJ):
                        w1k = wsl.tile([128, NJ * 128], BF16, tag="wsl")
                        nc.sync.dma_start(
                            w1k[:], di["W1"].ap()[l, k * 128:(k + 1) * 128,
                                                  g * D:(g + 1) * D])
                        for n in range(NJ):
                            nc.tensor.matmul(
                                psg[n][:], w1k[:, n * 128:(n + 1) * 128],
                                t_xn2[:, k * TPC:(k + 1) * TPC],
                                start=(k == 0), stop=(k == NJ - 1))
                    for n in range(NJ):
                        gn = g * NJ + n
                        nc.scalar.activation(
                            t_h1[:, gn * TPC:(gn + 1) * TPC], psg[n][:], AF.Gelu,
                            bias=t_b1[:, gn:gn + 1])

                p2_ = [psum() for _ in range(NJ)]
                for k in range(NJ1):
                    w2k = wsl.tile([128, NJ * 128], BF16, tag="wsl")
                    nc.sync.dma_start(w2k[:],
                                      di["W2"].ap()[l, k * 128:(k + 1) * 128, :])
                    for n in range(NJ):
                        nc.tensor.matmul(p2_[n][:], w2k[:, n * 128:(n + 1) * 128],
                                         t_h1[:, k * TPC:(k + 1) * TPC],
                                         start=(k == 0), stop=(k == NJ1 - 1))
                eb_2 = evict_bias(t_mo, lambda n: slice(n * TPC, (n + 1) * TPC),
                                  bt["b2_p"], 1)

                def mk_pre2(psums, eb, mo):
                    def pre2(j):
                        sl = slice(j * TPC, (j + 1) * TPC)
                        eb(j, psums[j])
                        nc.vector.tensor_tensor(t_x[:, sl], t_x[:, sl],
                                                mo[:, sl], OP.add)
                    return pre2
                pend[0] = mk_pre2(p2_, eb_2, t_mo)

        # ================= phase B: final LN + classifier =================
        with ExitStack() as bctx:
            bw = bctx.enter_context(tc.tile_pool(name="bw", bufs=1))
            hallp = bctx.enter_context(tc.tile_pool(name="hall", bufs=1))
            embp = bctx.enter_context(tc.tile_pool(name="embp", bufs=14))

            layernorm(bw, t_x, t_hT, t_lnw, t_lnb, pre=pend[0])
            pend[0] = None
            hag_in = dram.tile([D, TPC], BF16, tag="hag_in")
            nc.scalar.dma_start(
                hag_in[:].rearrange("(j p) t -> p j t", p=128),
                t_hT[:].rearrange("p (j t) -> p j t", j=NJ))
            hag_out = dram.tile([8 * D, TPC], BF16, tag="hag_out",
                                addr_space="Shared")
            nc.gpsimd.collective_compute(
                "AllGather", OP.bypass,
                replica_groups=[[0, 1, 2, 3, 4, 5, 6, 7]],
                ins=[hag_in[:].opt()], outs=[hag_out[:].opt()])

            t_hall = hallp.tile([128, 8 * NJ * TPC], BF16, tag="hall")
            hall4 = t_hall[:].rearrange("p (r j t) -> p r j t", r=8, j=NJ)
            nc.scalar.dma_start(
                hall4[:, 0:4], hag_out[0:4 * D, :]
                .rearrange("(r j p) t -> p r j t", r=4, p=128))
            nc.gpsimd.dma_start(
                hall4[:, 4:8], hag_out[4 * D:8 * D, :]
                .rearrange("(r j p) t -> p r j t", r=4, p=128))

            for vc in range(NVC):
                ets = []
                for k in range(NJ):
                    et = embp.tile([128, VCHUNK], BF16, tag="emb", name=f"emb{vc}_{k}")
                    nc.sync.dma_start(
                        et[:], di["embT"].ap()[k * 128:(k + 1) * 128,
                                               vc * VCHUNK:(vc + 1) * VCHUNK])
                    ets.append(et)
                for mt in range(NMT):
                    beta, j = divmod(mt, NB)
                    r, hf = beta * 4 + KB_RANK[j], KB_HALF[j]
                    pc = psum(VCHUNK)
                    for k in range(NJ):
                        nc.tensor.matmul(
                            pc[:],
                            t_hall[:, (r * NJ + k) * TPC + hf * TB:
                                   (r * NJ + k) * TPC + (hf + 1) * TB],
                            ets[k][:], start=(k == 0), stop=(k == NJ - 1))
                    so = embp.tile([128, VCHUNK], BF16, tag="clso",
                                   name=f"clso{vc}_{mt}")
                    if mt % 2 == 0:
                        nc.scalar.copy(so[:], pc[:])
                    else:
                        nc.vector.tensor_copy(so[:], pc[:])
                    nc.gpsimd.dma_start(
                        out_logits.ap()[mt * 128:(mt + 1) * 128,
                                        vc * VCHUNK:(vc + 1) * VCHUNK], so[:])

    nc.compile()
    return nc


_NC = None


def _get_nc():
    global _NC
    if _NC is None:
        _NC = _build()
    return _NC


def _pack_fm(M):
    """[768, t] feature-major -> [128, 6*t] tile layout (row d=128*j+p)."""
    t = M.shape[1]
    return np.ascontiguousarray(
        M.reshape(NJ, 128, t).transpose(1, 0, 2).reshape(128, NJ * t),
        dtype=np.float32)


def _pack_pp(v):
    """per-feature vector [D'] -> per-partition [128, D'/128]."""
    return np.ascontiguousarray(v.reshape(-1, 128).T, dtype=np.float32)


def _prep_in_maps(inputs):
    import ml_dtypes
    bf = ml_dtypes.bfloat16
    f32 = lambda a: np.ascontiguousarray(a, dtype=np.float32)
    f16 = lambda a: np.ascontiguousarray(a, dtype=bf)
    emb = f32(inputs["emb"])
    tok = np.asarray(inputs["input_token"]).astype(np.int64)
    x0 = emb[tok]                                    # [B, T, D]

    shared = {
        "Wq": f16(inputs["Wq"]), "Wk": f16(inputs["Wk"]),
        "Wv": f16(inputs["Wv"]), "Wo": f16(inputs["Wo"]),
        "W1": f16(inputs["W1"]), "W2": f16(inputs["W2"]),
        "onecol": np.full((128, 1), 1.0 / D, np.float32),
        "onesrow": np.ones((1, 128), np.float32),
        "ones96": np.ones((128, NB * H, 2), bf),
        "lnw_p": _pack_pp(f32(inputs["ln_w"])),
        "lnb_p": _pack_pp(f32(inputs["ln_b"])),
    }
    for nm, src in (("bq_p", "bq"), ("bk_p", "bk"), ("bo_p", "bo"),
                    ("b2_p", "b2"), ("g_p", "gamma"), ("be_p", "beta"),
                    ("l2w_p", "ln2_w"), ("l2b_p", "ln2_b")):
        shared[nm] = np.stack([_pack_pp(f32(inputs[src][l])) for l in range(L)])
    shared["b1_p"] = np.stack([_pack_pp(f32(inputs["b1"][l])) for l in range(L)])
    shared["bv_bc"] = np.stack(
        [np.tile(f32(inputs["bv"][l])[None, :], (128, 1)) for l in range(L)])

    # rope tables for one block-pair are built per core below
    inv = 1.0 / (10000.0 ** (np.arange(0, DK, 2, dtype=np.float32) / DK))
    embT_full = emb.T                                # [D, V]
    vpad = np.zeros((D, 8 * VC), np.float32)
    vpad[:, :V] = embT_full

    # diag causal mask (key-major): M[kt, qt] = 1 if kt <= qt
    diag = np.tril(np.ones((TB, TB), np.float32)).T

    in_maps = []
    for c in range(8):
        beta, i = divmod(c, 4)
        qb = (i, 7 - i)
        pos = np.concatenate([np.arange(qb[0] * TB, (qb[0] + 1) * TB),
                              np.arange(qb[1] * TB, (qb[1] + 1) * TB)])
        xc = x0[beta, pos]                           # [256, D]
        m = dict(shared)
        m["x0T"] = _pack_fm(xc.T)

        fr = pos[:, None].astype(np.float32) * inv[None, :]      # [256, 32]
        ang = np.concatenate([fr, fr], 1)                        # [256, 64]
        cosT = np.cos(ang).T                                     # [64, 256]
        sinT = np.sin(ang).T
        sinSg = sinT.copy()
        sinSg[:32] = -sinT[:32]
        m["cosT"] = np.ascontiguousarray(np.tile(cosT, (2, NJ))).astype(bf)
        m["sinS"] = np.ascontiguousarray(np.tile(sinSg, (2, NJ))).astype(bf)

        # block slots ordered (rank, half): slot 2r+f covers global block
        # r (f=0) or 7-r (f=1)
        masks = np.zeros((NB, 128, TPC), np.float32)
        for r4 in range(4):
            for f in range(2):
                kbg = r4 if f == 0 else 7 - r4
                slot = 2 * r4 + f
                for qi in range(2):
                    blk = qb[qi]
                    if kbg < blk:
                        masks[slot, :, qi * TB:(qi + 1) * TB] = 1.0
                    elif kbg == blk:
                        masks[slot, :, qi * TB:(qi + 1) * TB] = diag
        m["masks"] = masks.astype(bf)

        esl = np.zeros((D, VCP), np.float32)
        esl[:, :VC] = vpad[:, c * VC:(c + 1) * VC]
        m["embT"] = esl.astype(bf)
        in_maps.append(m)

    return in_maps


def _assemble(res):
    out = np.empty((B, T, 8 * VC), np.float32)
    for c in range(8):
        lr = np.asarray(res.results[c]["logits"]).astype(np.float32)
        lr = lr.reshape(B, T, VCP)
        out[:, :, c * VC:(c + 1) * VC] = lr[:, :, :VC]
    return np.ascontiguousarray(out[:, :, :V])


def kernel(**inputs):
    nc = _get_nc()
    in_maps = _prep_in_maps(inputs)
    res = bass_utils.run_bass_kernel_spmd(nc, in_maps, core_ids=list(range(8)))
    return _assemble(res)


def run_traced(inputs, tmpdir):
    nc = _get_nc()
    in_maps = _prep_in_maps(inputs)
    return bass_utils.run_bass_kernel_spmd(
        nc, in_maps, core_ids=list(range(8)), trace=True, tmpdir=tmpdir)


# revision 20
# speedup vs baseline: 1.1925x; 1.0125x over previous
"""GPT forward (L=6, B=2, T=1024, D=768, H=12, V=50257) on 8 TRN2 NeuronCores.

Sharding: tokens sharded 8-way (each core owns two causally-complementary
128-token blocks of one batch), weights replicated, per-layer K/V AllGather
within each 4-core batch group, classifier vocab-sharded 8-way after a final
hidden-state AllGather.  Activations are feature-major [D, t].

All matmul operands are bf16 (enables FWL fast weight load and full-rate
small matmuls); the residual stream and LN statistics stay fp32.  PSUM
evictions are spread across vector/gpsimd to keep the scalar engine free
for exp/gelu.  RoPE partition swaps run as SBUF->SBUF DMAs on four
different queues.  Collective payloads are bf16 (half the wire bytes).
The program is core-uniform: per-core differences (token positions, causal
masks, vocab slice) enter as input data.
"""
import os
import numpy as np
from contextlib import ExitStack

import concourse.bass as bass
import concourse.tile as tile
import concourse.mybir as mybir
from concourse import bacc, bass_utils
from concourse.masks import make_identity

F32 = mybir.dt.float32
F32R = mybir.dt.float32r
BF16 = mybir.dt.bfloat16
AF = mybir.ActivationFunctionType
OP = mybir.AluOpType

L, B, T, D, H, DK, V = 6, 2, 1024, 768, 12, 64, 50257
NB, TB, TPC = 8, 128, 256
NJ = D // 128                       # 6
NJ1 = 4 * D // 128                  # 24
VCHUNK = 512
NVC = 13
VCP = NVC * VCHUNK                  # 6656
VC = 6283                           # 8*6283 = 50264 >= V
EPS = 1e-5
NMT = 16
NLAYER = int(os.environ.get("KLAYERS", str(L)))

KB_RANK = [j if j < 4 else 7 - j for j in range(NB)]
KB_HALF = [0 if j < 4 else 1 for j in range(NB)]


def _build():
    nc = bacc.Bacc("TRN2", target_bir_lowering=False, debug=False)

    di = {}
    def din(name, shape, dt=F32R):
        di[name] = nc.dram_tensor(name, shape, dt, kind="ExternalInput")
        return di[name]

    din("x0T", [128, NJ * TPC])
    din("cosT", [128, NJ * TPC], BF16)
    din("sinS", [128, NJ * TPC], BF16)
    din("masks", [NB + 2, 128, TPC], BF16)
    din("onecol", [128, 1])
    din("onesrow", [1, 128])
    din("ones96", [128, (NB + 2) * H, 2], BF16)
    din("embT", [D, VCP], BF16)
    for nm in ("Wq", "Wk", "Wv", "Wo"):
        din(nm, [L, D, D], BF16)
    din("W1", [L, D, 4 * D], BF16)
    din("W2", [L, 4 * D, D], BF16)
    for nm in ("bq_p", "bk_p", "bo_p", "b2_p", "g_p", "be_p", "l2w_p", "l2b_p"):
        din(nm, [L, 128, NJ], F32)
    din("b1_p", [L, 128, NJ1], F32)
    din("bv_bc", [L, 128, D], F32)
    din("lnw_p", [128, NJ], F32)
    din("lnb_p", [128, NJ], F32)

    out_logits = nc.dram_tensor("logits", [NMT * 128, VCP], BF16,
                                kind="ExternalOutput")

    with tile.TileContext(nc) as tc, ExitStack() as octx:
        const = octx.enter_context(tc.tile_pool(name="const", bufs=1))
        xpool = octx.enter_context(tc.tile_pool(name="x", bufs=1))
        small = octx.enter_context(tc.tile_pool(name="small", bufs=2))
        bias = octx.enter_context(tc.tile_pool(name="bias", bufs=2))
        pp = octx.enter_context(tc.tile_pool(name="pp", bufs=8, space="PSUM"))
        dram = octx.enter_context(tc.tile_pool(name="dram", bufs=2, space="DRAM"))

        t_ones = const.tile([128, 1], F32R, tag="ones")
        nc.sync.dma_start(t_ones[:], di["onecol"].ap())
        t_onesr = const.tile([1, 128], F32R, tag="onesr")
        nc.sync.dma_start(t_onesr[:], di["onesrow"].ap())
        t_id = const.tile([128, 128], BF16, tag="ident")
        make_identity(nc, t_id[:])
        t_lnw = const.tile([128, NJ], F32, tag="lnw")
        nc.sync.dma_start(t_lnw[:], di["lnw_p"].ap())
        t_lnb = const.tile([128, NJ], F32, tag="lnb")
        nc.sync.dma_start(t_lnb[:], di["lnb_p"].ap())
        t_eps = const.tile([1, 1], F32, tag="eps")
        nc.gpsimd.memset(t_eps[:], EPS)

        t_x = xpool.tile([128, NJ * TPC], F32R, tag="x")
        nc.sync.dma_start(t_x[:], di["x0T"].ap())
        t_hT = xpool.tile([128, NJ * TPC], BF16, tag="hT")

        pcnt = [0]

        def psum(w=TPC):
            pcnt[0] += 1
            return pp.tile([128, w], F32, tag="pp", name=f"ps{pcnt[0]}")

        def psum1(w=TPC):
            pcnt[0] += 1
            return pp.tile([1, w], F32, tag="pp", name=f"ps{pcnt[0]}")

        def layernorm(wpool, src, dst, gt, bt, pre=None):
            """feature-major LN: dst(bf16) = (src - mean)/std * g + b.
            pre(j) runs before chunk j's stats (fused residual eviction)."""
            t_sq = wpool.tile([128, NJ * TPC], F32R, tag="scratch6")
            p_s = psum1()
            p_q = psum1()
            for j in range(NJ):
                if pre is not None:
                    pre(j)
                sl = slice(j * TPC, (j + 1) * TPC)
                nc.gpsimd.tensor_tensor(t_sq[:, sl], src[:, sl], src[:, sl],
                                        OP.mult)
                nc.tensor.matmul(p_s[:], t_ones[:], src[:, sl],
                                 start=(j == 0), stop=(j == NJ - 1))
                nc.tensor.matmul(p_q[:], t_ones[:], t_sq[:, sl],
                                 start=(j == 0), stop=(j == NJ - 1))
            # ones vector holds 1/D, so p_s = mean and p_q = E[x^2] directly
            t_mean = small.tile([1, TPC], F32R, tag="mean")
            nc.vector.tensor_copy(t_mean[:], p_s[:])
            t_msq = small.tile([1, TPC], F32, tag="msq")
            nc.vector.tensor_tensor(t_msq[:], t_mean[:], p_s[:], OP.mult)
            t_var = small.tile([1, TPC], F32, tag="var")
            nc.vector.tensor_tensor(t_var[:], p_q[:], t_msq[:], OP.subtract)
            t_std = small.tile([1, TPC], F32, tag="std")
            nc.scalar.activation(t_std[:], t_var[:], AF.Sqrt, bias=t_eps[:])
            t_rstd = small.tile([1, TPC], F32R, tag="rstd")
            with nc.allow_low_precision(reason="f32r view of f32 for PE bcast"):
                nc.vector.reciprocal(t_rstd[:], t_std[:])
            t_mb = small.tile([128, TPC], F32, tag="mb")
            p_mb = psum()
            nc.tensor.matmul(p_mb[:], t_onesr[:], t_mean[:])
            nc.vector.tensor_copy(t_mb[:], p_mb[:])
            t_rb = small.tile([128, TPC], F32, tag="rb")
            p_rb = psum()
            nc.tensor.matmul(p_rb[:], t_onesr[:], t_rstd[:])
            nc.vector.tensor_copy(t_rb[:], p_rb[:])
            t_c = wpool.tile([128, NJ * TPC], F32, tag="lnc")
            for j in range(NJ):
                sl = slice(j * TPC, (j + 1) * TPC)
                eng = nc.vector if j % 2 == 0 else nc.gpsimd
                eng.tensor_tensor(t_c[:, sl], src[:, sl], t_mb[:], OP.subtract)
                eng.tensor_tensor(dst[:, sl], t_c[:, sl], t_rb[:], OP.mult)
                eng.tensor_scalar(dst[:, sl], dst[:, sl], gt[:, j:j + 1],
                                  bt[:, j:j + 1], OP.mult, OP.add)

        def rope(wpool, t_q, t_cos, t_sin):
            """in-place RoPE on feature-major bf16 [128, NJ*TPC] tile.
            Partition swap via SBUF->SBUF DMAs on four queues."""
            t_sw = wpool.tile([128, NJ * TPC], BF16, tag="ropesw")
            W = NJ * TPC
            nc.sync.dma_start(t_sw[0:32, 0:W], t_q[32:64, 0:W])
            nc.scalar.dma_start(t_sw[32:64, 0:W], t_q[0:32, 0:W])
            nc.gpsimd.dma_start(t_sw[64:96, 0:W], t_q[96:128, 0:W])
            nc.sync.dma_start(t_sw[96:128, 0:W], t_q[64:96, 0:W])
            nc.vector.tensor_tensor(t_sw[:], t_sw[:], t_sin[:], OP.mult)
            nc.vector.tensor_tensor(t_q[:], t_q[:], t_cos[:], OP.mult)
            nc.vector.tensor_tensor(t_q[:], t_q[:], t_sw[:], OP.add)

        def wpass(wsl_pool, wdram, l, nk, rhs, rhs_k_slice, out_fn):
            """out[n] = sum_k W[l,k].T @ rhs_k ; W streamed, psum-resident over n.
            out_fn(n, ps) evicts psum tile for output feature-tile n."""
            pss = [psum() for _ in range(NJ)]
            for k in range(nk):
                wk = wsl_pool.tile([128, NJ * 128], BF16, tag="wsl")
                nc.sync.dma_start(wk[:], wdram.ap()[l, k * 128:(k + 1) * 128, :])
                for n in range(NJ):
                    nc.tensor.matmul(pss[n][:], wk[:, n * 128:(n + 1) * 128],
                                     rhs[:, rhs_k_slice(k)],
                                     start=(k == 0), stop=(k == nk - 1))
            for n in range(NJ):
                out_fn(n, pss[n])

        def evict_bias(dst, dst_sl, bias_t, flip=0):
            """psum + bias -> bf16 sbuf, alternating vector / scalar."""
            def f(nn, p):
                if (nn + flip) % 2 == 0:
                    nc.vector.tensor_scalar(dst[:, dst_sl(nn)], p[:],
                                            bias_t[:, nn:nn + 1], None, OP.add)
                else:
                    nc.scalar.activation(dst[:, dst_sl(nn)], p[:], AF.Identity,
                                         bias=bias_t[:, nn:nn + 1])
            return f

        pend = [None]
        # ================= phase A: transformer layers =================
        with ExitStack() as actx:
            aconst = actx.enter_context(tc.tile_pool(name="aconst", bufs=1))
            kvp = actx.enter_context(tc.tile_pool(name="kvp", bufs=1))
            wk_ = actx.enter_context(tc.tile_pool(name="work", bufs=1))
            ap_ = actx.enter_context(tc.tile_pool(name="Ap", bufs=12))
            wsl = actx.enter_context(tc.tile_pool(name="wsl", bufs=6))
            h1p = actx.enter_context(tc.tile_pool(name="h1p", bufs=1))

            t_cos = aconst.tile([128, NJ * TPC], BF16, tag="cos")
            nc.sync.dma_start(t_cos[:], di["cosT"].ap())
            t_sin = aconst.tile([128, NJ * TPC], BF16, tag="sin")
            nc.sync.dma_start(t_sin[:], di["sinS"].ap())
            t_mask = aconst.tile([128, (NB + 2) * TPC], BF16, tag="mask")
            for kb in range(NB + 2):
                nc.sync.dma_start(t_mask[:, kb * TPC:(kb + 1) * TPC],
                                  di["masks"].ap()[kb])

            NS = NB + 2      # 8 gathered slots + 2 local (own-block) slots
            t_K = kvp.tile([128, NJ * NS * TB], BF16, tag="K")    # (j, slot, t)
            t_V = kvp.tile([128, NS * H * 66], BF16, tag="V")     # (slot, h, dk|one)
            nc.sync.dma_start(
                t_V[:].rearrange("p (b h e) -> p (b h) e", b=NS, h=H)[:, :, 64:66],
                di["ones96"].ap())

            for l in range(NLAYER):
                # --- per-layer bias/param tiles
                bt = {}
                for nm in ("bq_p", "bk_p", "bo_p", "b2_p", "g_p", "be_p",
                           "l2w_p", "l2b_p"):
                    bt[nm] = bias.tile([128, NJ], F32, tag=nm, name=f"bt_{nm}")
                    nc.sync.dma_start(bt[nm][:], di[nm].ap()[l])
                t_b1 = bias.tile([128, NJ1], F32, tag="b1")
                nc.sync.dma_start(t_b1[:], di["b1_p"].ap()[l])
                t_bvb = bias.tile([128, D], F32, tag="bvb")
                nc.sync.dma_start(t_bvb[:], di["bv_bc"].ap()[l])

                # --- LN1 (fused with the previous layer's W2 eviction+residual)
                t_xn = wk_.tile([128, NJ * TPC], BF16, tag="xn")
                layernorm(wk_, t_x, t_xn, bt["g_p"], bt["be_p"],
                          pre=pend[0])
                pend[0] = None

                # --- K projection (feature-major), RoPE, AllGather
                t_k = wk_.tile([128, NJ * TPC], BF16, tag="k")
                wpass(wsl, di["Wk"], l, NJ, t_xn,
                      lambda k: slice(k * TPC, (k + 1) * TPC),
                      evict_bias(t_k, lambda n: slice(n * TPC, (n + 1) * TPC),
                                 bt["bk_p"], 0))
                rope(wk_, t_k, t_cos, t_sin)
                # --- stage K into the merged K+V collective buffer
                kv_in = dram.tile([2 * D * TPC], BF16, tag="kv_in")
                nc.scalar.dma_start(
                    kv_in[0:D * TPC].rearrange("(j p t) -> p j t", j=NJ, p=128),
                    t_k[:].rearrange("p (j t) -> p j t", j=NJ))
                # --- V projection (token-major) + bias
                t_vc = wk_.tile([128, 2 * D], BF16, tag="vc")
                psv = [[psum(512), psum(256)] for _ in range(2)]
                for k in range(NJ):
                    wvk = wsl.tile([128, NJ * 128], BF16, tag="wsl")
                    nc.sync.dma_start(wvk[:], di["Wv"].ap()[l, k * 128:(k + 1) * 128, :])
                    for tt in range(2):
                        lhs = t_xn[:, k * TPC + tt * TB: k * TPC + (tt + 1) * TB]
                        nc.tensor.matmul(psv[tt][0][:], lhs, wvk[:, 0:512],
                                         start=(k == 0), stop=(k == NJ - 1))
                        nc.tensor.matmul(psv[tt][1][:], lhs, wvk[:, 512:768],
                                         start=(k == 0), stop=(k == NJ - 1))
                for tt in range(2):
                    nc.vector.tensor_tensor(
                        t_vc[:, tt * D: tt * D + 512], psv[tt][0][:],
                        t_bvb[:, 0:512], OP.add)
                    nc.vector.tensor_tensor(
                        t_vc[:, tt * D + 512: (tt + 1) * D], psv[tt][1][:],
                        t_bvb[:, 512:768], OP.add)

                nc.scalar.dma_start(
                    kv_in[D * TPC:].rearrange("(tt p e) -> p tt e", tt=2, p=128),
                    t_vc[:].rearrange("p (tt e) -> p tt e", tt=2))
                vvo = t_V[:].rearrange("p (b h e) -> p b h e", b=NB + 2, h=H)
                for f in range(2):
                    nc.sync.dma_start(
                        vvo[:, NB + f, :, 0:64],
                        t_vc[:, f * D:(f + 1) * D]
                        .rearrange("p (h e) -> p h e", h=H))
                kv_out = dram.tile([4, 2 * D * TPC], BF16, tag="kv_out")
                nc.gpsimd.collective_compute(
                    "AllGather", OP.bypass,
                    replica_groups=[[0, 1, 2, 3], [4, 5, 6, 7]],
                    ins=[kv_in[:].opt()], outs=[kv_out[:].opt()])

                # --- Q projection + RoPE (overlaps the K/V collectives)
                t_q = wk_.tile([128, NJ * TPC], BF16, tag="q")
                wpass(wsl, di["Wq"], l, NJ, t_xn,
                      lambda k: slice(k * TPC, (k + 1) * TPC),
                      evict_bias(t_q, lambda n: slice(n * TPC, (n + 1) * TPC),
                                 bt["bq_p"], 1))
                rope(wk_, t_q, t_cos, t_sin)

                # --- attention: head pairs share the PE via row groups.
                # Slots 8,9 hold this core's own K/V (read pre-gather): their
                # QK/exp runs during the collective; per-core masks zero the
                # duplicated gathered slots.
                t_att = wk_.tile([128, 2 * D], BF16, tag="att")   # (qi, h, dk)
                tAs = [[ap_.tile([128, NS * TPC], BF16, tag="A",
                                 name=f"A{l}_{hp}_{i}") for i in range(2)]
                       for hp in range(H // 2)]

                def qk_chunk(hp, kbp, ksrc, kcol):
                    """QK for slot pair kbp (cols from ksrc at kcol), both
                    heads of pair hp, then exp+mask into tAs[hp]."""
                    jq = hp
                    ps2 = [psum(512) for _ in range(2)]
                    for kk in range(2):
                        for hi in range(2):
                            po = 64 * hi
                            nc.tensor.matmul(
                                ps2[hi][:, kk * TPC:(kk + 1) * TPC],
                                ksrc[po:po + 64,
                                     kcol + kk * TB: kcol + (kk + 1) * TB],
                                t_q[po:po + 64, jq * TPC:(jq + 1) * TPC])
                    for hi in range(2):
                        asl = tAs[hp][hi][:, kbp * 512:(kbp + 1) * 512]
                        nc.scalar.activation(asl, ps2[hi][:], AF.Exp,
                                             scale=0.125)
                        eng = (nc.vector if (kbp + hi) % 2 == 0 or kbp == 4
                               else nc.gpsimd)
                        eng.tensor_tensor(
                            asl, asl, t_mask[:, kbp * 512:(kbp + 1) * 512],
                            OP.mult)

                for hp in range(H // 2):     # own slots first (no AG dep)
                    qk_chunk(hp, 4, t_k, hp * TPC)
                # --- load gathered K (feature-major) and V (token-major)
                # block slot order is (rank, half); masks are relabeled to match
                kk5 = t_K[:].rearrange("p (j b t) -> p j b t", j=NJ, b=NS)
                vv5 = t_V[:].rearrange("p (b h e) -> p b h e", b=NS, h=H)
                for r in range(4):
                    dk = kk5[:, :, 2 * r:2 * r + 2, :]
                    nc.scalar.dma_start(
                        dk.rearrange("p j b t -> p j (b t)"),
                        kv_out[r, 0:D * TPC]
                        .rearrange("(j p t) -> p j t", j=NJ, p=128))
                    for f in range(2):
                        nc.gpsimd.dma_start(
                            vv5[:, 2 * r + f, :, 0:64],
                            kv_out[r, D * TPC + f * (TB * D):
                                   D * TPC + (f + 1) * (TB * D)]
                            .rearrange("(p h e) -> p h e", p=128, h=H))

                for hp in range(H // 2):     # gathered slots
                    for kbp in range(4):
                        qk_chunk(hp, kbp, t_K, (hp * NS + 2 * kbp) * TB)
                for hp in range(H // 2):
                    for hi in range(2):
                        h = 2 * hp + hi
                        for qi in range(2):
                            pav = psum(66)
                            for kb in range(NS):
                                nc.tensor.matmul(
                                    pav[:],
                                    tAs[hp][hi][:, kb * TPC + qi * TB:
                                                kb * TPC + (qi + 1) * TB],
                                    t_V[:, (kb * H + h) * 66:(kb * H + h) * 66 + 66],
                                    start=(kb == 0), stop=(kb == NS - 1))
                            t_rl = small.tile([128, 1], F32, tag="rl")
                            nc.vector.reciprocal(t_rl[:], pav[:, 64:65])
                            osl = t_att[:, qi * D + h * 64: qi * D + (h + 1) * 64]
                            if (hi + qi) % 2 == 0:
                                nc.vector.tensor_scalar_mul(osl, pav[:, 0:64],
                                                            t_rl[:])
                            else:
                                nc.scalar.mul(osl, pav[:, 0:64], t_rl[:])

                # --- transpose att to feature-major
                t_attT = wk_.tile([128, NJ * TPC], BF16, tag="attT")
                for qi in range(2):
                    for j in range(NJ):
                        pcnt[0] += 1
                        ptr = pp.tile([128, 128], BF16, tag="pp",
                                      name=f"ps{pcnt[0]}")
                        nc.tensor.transpose(
                            ptr[:], t_att[:, qi * D + j * 128: qi * D + (j + 1) * 128],
                            t_id[:])
                        osl = t_attT[:, j * TPC + qi * TB: j * TPC + qi * TB + TB]
                        if (qi + j) % 2 == 0:
                            nc.vector.tensor_copy(osl, ptr[:])
                        else:
                            nc.scalar.copy(osl, ptr[:])

                # --- Wo + residual fused into LN2 chunk prologue
                t_mo = wk_.tile([128, NJ * TPC], F32, tag="mmout")
                po_ = [psum() for _ in range(NJ)]
                for k in range(NJ):
                    wok = wsl.tile([128, NJ * 128], BF16, tag="wsl")
                    nc.sync.dma_start(wok[:],
                                      di["Wo"].ap()[l, k * 128:(k + 1) * 128, :])
                    for n in range(NJ):
                        nc.tensor.matmul(po_[n][:], wok[:, n * 128:(n + 1) * 128],
                                         t_attT[:, k * TPC:(k + 1) * TPC],
                                         start=(k == 0), stop=(k == NJ - 1))
                eb_o = evict_bias(t_mo, lambda n: slice(n * TPC, (n + 1) * TPC),
                                  bt["bo_p"], 0)

                def pre_o(j):
                    sl = slice(j * TPC, (j + 1) * TPC)
                    eb_o(j, po_[j])
                    nc.vector.tensor_tensor(t_x[:, sl], t_x[:, sl], t_mo[:, sl],
                                            OP.add)

                # --- LN2 + MLP
                t_xn2 = wk_.tile([128, NJ * TPC], BF16, tag="xn")
                layernorm(wk_, t_x, t_xn2, bt["l2w_p"], bt["l2b_p"], pre=pre_o)

                t_h1 = h1p.tile([128, NJ1 * TPC], BF16, tag="h1")
                for g in range(4):
                    psg = [psum() for _ in range(NJ)]
                    for k in range(NJ):
                        w1k = wsl.tile([128, NJ * 128], BF16, tag="wsl")
                        nc.sync.dma_start(
                            w1k[:], di["W1"].ap()[l, k * 128:(k + 1) * 128,
                                                  g * D:(g + 1) * D])
                        for n in range(NJ):
                            nc.tensor.matmul(
                                psg[n][:], w1k[:, n * 128:(n + 1) * 128],
                                t_xn2[:, k * TPC:(k + 1) * TPC],
                                start=(k == 0), stop=(k == NJ - 1))
                    for n in range(NJ):
                        gn = g * NJ + n
                        nc.scalar.activation(
                            t_h1[:, gn * TPC:(gn + 1) * TPC], psg[n][:], AF.Gelu,
                            bias=t_b1[:, gn:gn + 1])

                p2_ = [psum() for _ in range(NJ)]
                for k in range(NJ1):
                    w2k = wsl.tile([128, NJ * 128], BF16, tag="wsl")
                    nc.sync.dma_start(w2k[:],
                                      di["W2"].ap()[l, k * 128:(k + 1) * 128, :])
                    for n in range(NJ):
                        nc.tensor.matmul(p2_[n][:], w2k[:, n * 128:(n + 1) * 128],
                                         t_h1[:, k * TPC:(k + 1) * TPC],
                                         start=(k == 0), stop=(k == NJ1 - 1))
                eb_2 = evict_bias(t_mo, lambda n: slice(n * TPC, (n + 1) * TPC),
                                  bt["b2_p"], 1)

                def mk_pre2(psums, eb, mo):
                    def pre2(j):
                        sl = slice(j * TPC, (j + 1) * TPC)
                        eb(j, psums[j])
                        nc.vector.tensor_tensor(t_x[:, sl], t_x[:, sl],
                                                mo[:, sl], OP.add)
                    return pre2
                pend[0] = mk_pre2(p2_, eb_2, t_mo)

        # ================= phase B: final LN + classifier =================
        with ExitStack() as bctx:
            bw = bctx.enter_context(tc.tile_pool(name="bw", bufs=1))
            hallp = bctx.enter_context(tc.tile_pool(name="hall", bufs=1))
            embp = bctx.enter_context(tc.tile_pool(name="embp", bufs=14))

            layernorm(bw, t_x, t_hT, t_lnw, t_lnb, pre=pend[0])
            pend[0] = None
            hag_in = dram.tile([D, TPC], BF16, tag="hag_in")
            nc.scalar.dma_start(
                hag_in[:].rearrange("(j p) t -> p j t", p=128),
                t_hT[:].rearrange("p (j t) -> p j t", j=NJ))
            hag_out = dram.tile([8 * D, TPC], BF16, tag="hag_out",
                                addr_space="Shared")
            nc.gpsimd.collective_compute(
                "AllGather", OP.bypass,
                replica_groups=[[0, 1, 2, 3, 4, 5, 6, 7]],
                ins=[hag_in[:].opt()], outs=[hag_out[:].opt()])

            t_hall = hallp.tile([128, 8 * NJ * TPC], BF16, tag="hall")
            hall4 = t_hall[:].rearrange("p (r j t) -> p r j t", r=8, j=NJ)
            nc.scalar.dma_start(
                hall4[:, 0:4], hag_out[0:4 * D, :]
                .rearrange("(r j p) t -> p r j t", r=4, p=128))
            nc.gpsimd.dma_start(
                hall4[:, 4:8], hag_out[4 * D:8 * D, :]
                .rearrange("(r j p) t -> p r j t", r=4, p=128))

            for vc in range(NVC):
                ets = []
                for k in range(NJ):
                    et = embp.tile([128, VCHUNK], BF16, tag="emb", name=f"emb{vc}_{k}")
                    nc.sync.dma_start(
                        et[:], di["embT"].ap()[k * 128:(k + 1) * 128,
                                               vc * VCHUNK:(vc + 1) * VCHUNK])
                    ets.append(et)
                for mt in range(NMT):
                    beta, j = divmod(mt, NB)
                    r, hf = beta * 4 + KB_RANK[j], KB_HALF[j]
                    pc = psum(VCHUNK)
                    for k in range(NJ):
                        nc.tensor.matmul(
                            pc[:],
                            t_hall[:, (r * NJ + k) * TPC + hf * TB:
                                   (r * NJ + k) * TPC + (hf + 1) * TB],
                            ets[k][:], start=(k == 0), stop=(k == NJ - 1))
                    so = embp.tile([128, VCHUNK], BF16, tag="clso",
                                   name=f"clso{vc}_{mt}")
                    if mt % 2 == 0:
                        nc.scalar.copy(so[:], pc[:])
                    else:
                        nc.vector.tensor_copy(so[:], pc[:])
                    nc.gpsimd.dma_start(
                        out_logits.ap()[mt * 128:(mt + 1) * 128,
                                        vc * VCHUNK:(vc + 1) * VCHUNK], so[:])

    nc.compile()
    return nc


_NC = None


def _get_nc():
    global _NC
    if _NC is None:
        _NC = _build()
    return _NC


def _pack_fm(M):
    """[768, t] feature-major -> [128, 6*t] tile layout (row d=128*j+p)."""
    t = M.shape[1]
    return np.ascontiguousarray(
        M.reshape(NJ, 128, t).transpose(1, 0, 2).reshape(128, NJ * t),
        dtype=np.float32)


def _pack_pp(v):
    """per-feature vector [D'] -> per-partition [128, D'/128]."""
    return np.ascontiguousarray(v.reshape(-1, 128).T, dtype=np.float32)


def _prep_in_maps(inputs):
    import ml_dtypes
    bf = ml_dtypes.bfloat16
    f32 = lambda a: np.ascontiguousarray(a, dtype=np.float32)
    f16 = lambda a: np.ascontiguousarray(a, dtype=bf)
    emb = f32(inputs["emb"])
    tok = np.asarray(inputs["input_token"]).astype(np.int64)
    x0 = emb[tok]                                    # [B, T, D]

    shared = {
        "Wq": f16(inputs["Wq"]), "Wk": f16(inputs["Wk"]),
        "Wv": f16(inputs["Wv"]), "Wo": f16(inputs["Wo"]),
        "W1": f16(inputs["W1"]), "W2": f16(inputs["W2"]),
        "onecol": np.full((128, 1), 1.0 / D, np.float32),
        "onesrow": np.ones((1, 128), np.float32),
        "ones96": np.ones((128, (NB + 2) * H, 2), bf),
        "lnw_p": _pack_pp(f32(inputs["ln_w"])),
        "lnb_p": _pack_pp(f32(inputs["ln_b"])),
    }
    for nm, src in (("bq_p", "bq"), ("bk_p", "bk"), ("bo_p", "bo"),
                    ("b2_p", "b2"), ("g_p", "gamma"), ("be_p", "beta"),
                    ("l2w_p", "ln2_w"), ("l2b_p", "ln2_b")):
        shared[nm] = np.stack([_pack_pp(f32(inputs[src][l])) for l in range(L)])
    shared["b1_p"] = np.stack([_pack_pp(f32(inputs["b1"][l])) for l in range(L)])
    shared["bv_bc"] = np.stack(
        [np.tile(f32(inputs["bv"][l])[None, :], (128, 1)) for l in range(L)])

    # rope tables for one block-pair are built per core below
    inv = 1.0 / (10000.0 ** (np.arange(0, DK, 2, dtype=np.float32) / DK))
    embT_full = emb.T                                # [D, V]
    vpad = np.zeros((D, 8 * VC), np.float32)
    vpad[:, :V] = embT_full

    # diag causal mask (key-major): M[kt, qt] = 1 if kt <= qt
    diag = np.tril(np.ones((TB, TB), np.float32)).T

    in_maps = []
    for c in range(8):
        beta, i = divmod(c, 4)
        qb = (i, 7 - i)
        pos = np.concatenate([np.arange(qb[0] * TB, (qb[0] + 1) * TB),
                              np.arange(qb[1] * TB, (qb[1] + 1) * TB)])
        xc = x0[beta, pos]                           # [256, D]
        m = dict(shared)
        m["x0T"] = _pack_fm(xc.T)

        fr = pos[:, None].astype(np.float32) * inv[None, :]      # [256, 32]
        ang = np.concatenate([fr, fr], 1)                        # [256, 64]
        cosT = np.cos(ang).T                                     # [64, 256]
        sinT = np.sin(ang).T
        sinSg = sinT.copy()
        sinSg[:32] = -sinT[:32]
        m["cosT"] = np.ascontiguousarray(np.tile(cosT, (2, NJ))).astype(bf)
        m["sinS"] = np.ascontiguousarray(np.tile(sinSg, (2, NJ))).astype(bf)

        # slots 0-7 ordered (rank, half): slot 2r+f covers global block
        # r (f=0) or 7-r (f=1).  Slots 8,9 are this core's own two blocks
        # (computed pre-gather); the duplicated gathered slots are zeroed.
        masks = np.zeros((NB + 2, 128, TPC), np.float32)
        my_r = i
        for r4 in range(4):
            for f in range(2):
                kbg = r4 if f == 0 else 7 - r4
                slot = 2 * r4 + f
                if r4 == my_r:
                    continue                     # own block: handled in 8/9
                for qi in range(2):
                    blk = qb[qi]
                    if kbg < blk:
                        masks[slot, :, qi * TB:(qi + 1) * TB] = 1.0
                    elif kbg == blk:
                        masks[slot, :, qi * TB:(qi + 1) * TB] = diag
        for f in range(2):
            kbg = qb[f]
            for qi in range(2):
                blk = qb[qi]
                if kbg < blk:
                    masks[NB + f, :, qi * TB:(qi + 1) * TB] = 1.0
                elif kbg == blk:
                    masks[NB + f, :, qi * TB:(qi + 1) * TB] = diag
        m["masks"] = masks.astype(bf)

        esl = np.zeros((D, VCP), np.float32)
        esl[:, :VC] = vpad[:, c * VC:(c + 1) * VC]
        m["embT"] = esl.astype(bf)
        in_maps.append(m)

    return in_maps


def _assemble(res):
    out = np.empty((B, T, 8 * VC), np.float32)
    for c in range(8):
        lr = np.asarray(res.results[c]["logits"]).astype(np.float32)
        lr = lr.reshape(B, T, VCP)
        out[:, :, c * VC:(c + 1) * VC] = lr[:, :, :VC]
    return np.ascontiguousarray(out[:, :, :V])


def kernel(**inputs):
    nc = _get_nc()
    in_maps = _prep_in_maps(inputs)
    res = bass_utils.run_bass_kernel_spmd(nc, in_maps, core_ids=list(range(8)))
    return _assemble(res)


def run_traced(inputs, tmpdir):
    nc = _get_nc()
    in_maps = _prep_in_maps(inputs)
    return bass_utils.run_bass_kernel_spmd(
        nc, in_maps, core_ids=list(range(8)), trace=True, tmpdir=tmpdir)
